# revision 28
# baseline (speedup 1.0000x reference)
"""Causal multi-head attention for TRN2, sharded across 8 NeuronCores.

Problem: x[4,2048,1024] -> 16-head causal self-attention (head_dim 64) with
QKV + output projections, fp32.

Sharding: core c -> batch b = c // 2, head-group g = c % 2 (heads g*8..g*8+7).
Per core: Q/K/V projections use the 512 weight columns of its head-group
(column-parallel); attention runs over its 8 heads; the output projection
uses the matching 512 rows of wo (row-parallel), so each core emits a
partial [2048,1024] output and the host sums the two partials per batch.
bo is added on the g==0 cores only (g==1 cores receive zeros).

v2 changes over the 303us baseline:
  - Q/K projections run fp8e4 DoubleRow (contraction 256/matmul, half the
    matmul count). Weights are host-scaled x32 so they sit in fp8's normal
    range; the x1024 score scale folds into the exp's scale (2^-13). fp8
    noise here only perturbs softmax weights (~2%) which averages out in
    the AV reduction; V and the output projection stay bf16 because value-
    path fp8 error would land directly on the output.
  - bq/bk are never applied: softmax over k is invariant to the q-only
    terms Q.bk and bq.bk; the remaining bq.K term perturbs weights ~1%
    (rel-err budget is 2e-2). The PSUM->SBUF moves become plain copies.
  - Host ships j-chunk-major layouts so chunk-0 inputs land first; warmup
    matmuls start immediately so the PE HAM clock gate opens before the
    first projections; KTz/V padding via on-chip memset, not DRAM DMAs.
  - Score k-tiles are processed in pairs sharing a [128,2,512] PSUM tile;
    one ACTIVATE covers both (stale-PSUM columns in the not-computed
    diagonal range exp to junk that AV never reads), halving ACT
    instruction + semaphore count.
  - Keep-warm dummy matmuls cover the final softmax-normalize chain so the
    output-projection tail doesn't start HAM-cold.
"""

import os
from contextlib import ExitStack

import numpy as np

import concourse.bacc as bacc
import concourse.mybir as mybir
import concourse.tile as tile
from concourse.bass_utils import run_bass_kernel_spmd
from concourse.masks import make_upper_triangular

F32 = mybir.dt.float32
BF16 = mybir.dt.bfloat16
F8 = mybir.dt.float8e4
I16 = mybir.dt.int16
AF = mybir.ActivationFunctionType
ALU = mybir.AluOpType
DR = mybir.MatmulPerfMode.DoubleRow

B = 4
S = 2048
D = 1024
HD = 64
HG = 8  # heads per core
QC = HG * HD  # 512 local q/k/v columns
N_CORES = 8
WSCALE = 32.0  # host scale on wq/wk so fp8 stays in normal range
SC = 0.125 / (WSCALE * WSCALE)  # exp scale: 1/sqrt(HD) / (32*32) = 2^-13
# DVE fast-exp: bf16 bits of exp(s*SC) ~ int16(s*AEXP + BEXP); one
# tensor_scalar (mult,add with int16-convert output) per pair. Max rel
# error ~3.4% on softmax weights of the offloaded pairs.
AEXP = float(128.0 / np.log(2.0) * SC)
BEXP = 127.0 * 128.0 - 5.25

_NC_CACHE = {}
LAST_RESULT = None  # BassKernelResults of the most recent kernel() call


def _build_nc(s: int = S, num_devices: int = N_CORES):
    P = 128
    NQ = s // 512  # 512-col q-chunks
    NS = s // P  # 128-row s-tiles
    ND = D // P  # bf16 contraction tiles
    NC = D // 256  # fp8 DoubleRow contraction tiles
    NT = QC // P  # 128-row tiles of the local q/k/v columns
    VW = HD + 1  # 65: per-head V block width (64 cols + ones col)
    VPAD = 7 * VW + P  # 583: last head's lhsT slice must fit

    nc = bacc.Bacc("TRN2", target_bir_lowering=False, debug=False, num_devices=num_devices)

    xf8_d = nc.dram_tensor("xf8", [P, NQ * NC * 2 * 512], F8, kind="ExternalInput").ap()
    xbf_d = nc.dram_tensor("xbf", [P, NQ * ND * 512], BF16, kind="ExternalInput").ap()
    wqf8_d = nc.dram_tensor("wqf8", [P, NC * 2 * QC], F8, kind="ExternalInput").ap()
    wkf8_d = nc.dram_tensor("wkf8", [P, NC * 2 * QC], F8, kind="ExternalInput").ap()
    wv_d = nc.dram_tensor("wv", [P, ND * QC], BF16, kind="ExternalInput").ap()
    wo_d = nc.dram_tensor("wo", [P, NT * D], BF16, kind="ExternalInput").ap()
    bq32_d = nc.dram_tensor("bq32", [QC], F32, kind="ExternalInput").ap()
    bvb_d = nc.dram_tensor("bvb", [P, QC], F32, kind="ExternalInput").ap()
    bob_d = nc.dram_tensor("bob", [P, D], F32, kind="ExternalInput").ap()
    out_d = nc.dram_tensor("out", [s, D], F32, kind="ExternalOutput").ap()

    xf8_r = xf8_d.rearrange("p (j c i s) -> p j c i s", j=NQ, c=NC, i=2)
    xbf_r = xbf_d.rearrange("p (j d s) -> p j d s", j=NQ, d=ND)

    with tile.TileContext(nc) as tc:
        with ExitStack() as ctx:
            consts = ctx.enter_context(tc.tile_pool(name="consts", bufs=1))
            persist = ctx.enter_context(tc.tile_pool(name="persist", bufs=1))
            e_pool = ctx.enter_context(tc.tile_pool(name="epool", bufs=3))
            n_pool = ctx.enter_context(tc.tile_pool(name="npool", bufs=2))
            b_pool = ctx.enter_context(tc.tile_pool(name="bpool", bufs=4))
            o_pool = ctx.enter_context(tc.tile_pool(name="opool", bufs=3))
            proj_psum = ctx.enter_context(tc.tile_pool(name="proj_ps", bufs=2, space="PSUM"))
            s_psum = ctx.enter_context(tc.tile_pool(name="s_ps", bufs=2, space="PSUM"))
            a_psum = ctx.enter_context(tc.tile_pool(name="a_ps", bufs=2, space="PSUM"))

            # --- dep-free PE warmup first: matmuls on a memset junk tile
            # open the HAM clock gate while the DMAs stream in
            junk = consts.tile([P, P], BF16)
            nc.gpsimd.memset(junk[:], 0.5)
            warm = s_psum.tile([P, 2, 512], F32, tag="s", name="warm")
            for _ in range(64):
                nc.tensor.matmul(
                    warm[:, 0, 0:P], lhsT=junk[:], rhs=junk[:], start=True, stop=True
                )

            # Q/K live twice: straight (even head rows 0:64, odd 64:128 per
            # 128-col tile) and half-swapped, so the two k-tiles of a score
            # pair can run as concurrent K=64 row-tiled matmuls on the top
            # and bottom halves of the PE array (measured 2x).
            QT = persist.tile([P, NT, s], BF16)
            QTs = persist.tile([P, NT, s], BF16)
            KT = persist.tile([P, NT, s], BF16)
            KTs = persist.tile([P, NT, s], BF16)
            V = persist.tile([P, NS, VPAD + 1], BF16)
            AT = persist.tile([P, NT, s], BF16)

            # V ones columns (denominator trick); V's tail padding is left as
            # junk: it only feeds never-read PSUM rows (65+) of the AV accum.
            nc.gpsimd.memset(
                V[:, :, 0 : HG * VW].rearrange("p s (h c) -> p s h c", c=VW)[:, :, :, HD : HD + 1],
                1.0,
            )

            tri = consts.tile([P, P], F32)
            make_upper_triangular(nc, tri[:], val=1.0, diag=True)
            tri_b = consts.tile([P, P], BF16)
            nc.vector.tensor_copy(tri_b[:], tri[:])

            # --- inputs in order of first use; DMA issue is ~0.7us each on
            # the sync queue so first-needed must go first
            wqf8_sb = persist.tile([P, NC, 2, QC], F8)
            wkf8_sb = persist.tile([P, NC, 2, QC], F8)
            xf8_sb = persist.tile([P, NQ, NC, 2, 512], F8)
            xbf_sb = persist.tile([P, NQ, ND, 512], BF16)
            wv_sb = persist.tile([P, ND, QC], BF16)
            wo_sb = persist.tile([P, NT, D], BF16)
            bqc = consts.tile([P, NT], F32)
            bvb = consts.tile([P, QC], F32)
            bob = consts.tile([P, D], F32)

            nc.sync.dma_start(wqf8_sb[:], wqf8_d.rearrange("p (c i m) -> p c i m", c=NC, i=2))
            nc.sync.dma_start(xf8_sb[:, 0], xf8_r[:, 0])
            nc.sync.dma_start(wkf8_sb[:], wkf8_d.rearrange("p (c i m) -> p c i m", c=NC, i=2))
            nc.sync.dma_start(bqc[:], bq32_d.rearrange("(t p) -> p t", p=P))
            nc.sync.dma_start(xbf_sb[:, 0], xbf_r[:, 0])
            nc.sync.dma_start(wv_sb[:], wv_d.rearrange("p (d m) -> p d m", d=ND))
            nc.sync.dma_start(bvb[:], bvb_d)
            nc.sync.dma_start(bob[:], bob_d)
            for j in range(1, NQ):
                nc.sync.dma_start(xf8_sb[:, j], xf8_r[:, j])
            nc.sync.dma_start(wo_sb[:], wo_d.rearrange("p (t e) -> p t e", t=NT))
            for j in range(1, NQ):
                nc.sync.dma_start(xbf_sb[:, j], xbf_r[:, j])

            def proj_group(j, g):
                """One psum-group of the j-chunk projections; g in 0..11."""
                js = slice(j * 512, (j + 1) * 512)
                kind, t = divmod(g, NT)
                ps = proj_psum.tile([P, 512], F32, tag="pp", name="pp")
                if kind == 0:  # Q (fp8 DoubleRow)
                    for c in range(NC):
                        nc.tensor.matmul(
                            ps[:],
                            lhsT=wqf8_sb[:, c, :, t * P : (t + 1) * P],
                            rhs=xf8_sb[:, j, c],
                            start=(c == 0),
                            stop=(c == NC - 1),
                            perf_mode=DR,
                        )
                    nc.vector.tensor_scalar_add(QT[:, t, js], ps[:], bqc[:, t : t + 1])
                    if t == NT - 1:  # chunk's QT done: build the swapped copy
                        nc.sync.dma_start(QTs[0:64, :, js], QT[64:128, :, js])
                        nc.sync.dma_start(QTs[64:128, :, js], QT[0:64, :, js])
                elif kind == 1:  # K (fp8 DoubleRow)
                    for c in range(NC):
                        nc.tensor.matmul(
                            ps[:],
                            lhsT=wkf8_sb[:, c, :, t * P : (t + 1) * P],
                            rhs=xf8_sb[:, j, c],
                            start=(c == 0),
                            stop=(c == NC - 1),
                            perf_mode=DR,
                        )
                    nc.vector.tensor_copy(KT[:, t, js], ps[:])
                    if t == NT - 1:
                        nc.sync.dma_start(KTs[0:64, :, js], KT[64:128, :, js])
                        nc.sync.dma_start(KTs[64:128, :, js], KT[0:64, :, js])
                else:  # V s-tile 4j+t (bf16)
                    st = 4 * j + t
                    for d in range(ND):
                        nc.tensor.matmul(
                            ps[:],
                            lhsT=xbf_sb[:, j, d, t * P : (t + 1) * P],
                            rhs=wv_sb[:, d, :],
                            start=(d == 0),
                            stop=(d == ND - 1),
                        )
                    dst = V[:, st, 0 : HG * VW].rearrange("p (h c) -> p h c", c=VW)[:, :, 0:HD]
                    src = ps.rearrange("p (h c) -> p h c", c=HD)
                    bsrc = bvb.rearrange("p (h c) -> p h c", c=HD)
                    nc.vector.tensor_tensor(dst, src, bsrc, ALU.add)

            def attn_head(j, h):
                t, half = h // 2, h % 2
                pb = 64 * half
                nkb = 4 * j + 4
                A_ps = a_psum.tile([P, 512], F32, tag="A", name="A")

                # head's data sits in the top half of (KT, QT) for even
                # heads, of the swapped copies for odd heads; the pair's
                # second k-tile reads the other pair of the same data from
                # the bottom half so both matmuls run concurrently (K=64
                # row tiles at base partitions 0 and 64)
                if half == 0:
                    top, bot = (KT, QT), (KTs, QTs)
                else:
                    top, bot = (KTs, QTs), (KT, QT)

                def issue_scores(pr, offload=False):
                    sp = s_psum.tile([P, 2, 512], F32, tag="s", name="sp")
                    E = e_pool.tile([P, 2, 512], BF16, name="E")
                    y0s = [max(0, P * (2 * pr + m - 4 * j)) for m in range(2)]
                    for m in range(2):
                        kb = 2 * pr + m
                        K_, Q_ = (top, bot)[m]
                        pl = slice(0, 64) if m == 0 else slice(64, 128)
                        nc.tensor.matmul(
                            sp[:, m, y0s[m] :],
                            lhsT=K_[pl, t, kb * P : (kb + 1) * P],
                            rhs=Q_[pl, t, j * 512 + y0s[m] : (j + 1) * 512],
                            start=True,
                            stop=True,
                        )
                    if offload:
                        # off-diagonal pair: approximate exp on the vector
                        # engine (ACT is the bottleneck in late chunks)
                        nc.vector.tensor_scalar(
                            E[:].bitcast(I16), sp[:], AEXP, BEXP, op0=ALU.mult, op1=ALU.add
                        )
                    else:
                        # one exp for the pair; the [y0s[0]:y0s[1]) slice of
                        # the second half exps stale junk that AV never reads
                        nc.scalar.activation(
                            E[:, :, y0s[0] :], sp[:, :, y0s[0] :], AF.Exp, scale=SC
                        )
                    return E, y0s

                def issue_av(pr, E, y0s):
                    for m in range(2):
                        kb = 2 * pr + m
                        if kb >= 4 * j:
                            ym = y0s[m]
                            nc.vector.tensor_tensor(
                                E[:, m, ym : ym + P], E[:, m, ym : ym + P], tri_b[:], ALU.mult
                            )
                        nc.tensor.matmul(
                            A_ps[:, y0s[m] :],
                            lhsT=V[:, kb, h * VW : h * VW + P],
                            rhs=E[:, m, y0s[m] :],
                            start=(kb == 0),
                            stop=(kb == nkb - 1),
                        )

                # software pipeline: scores of pair p+1 issue before AV of
                # pair p, so the PE streams scores while the exp runs
                offl = {1: (), 2: (1,), 3: (1, 3)}.get(j, ())
                prev = None
                for pr in range(nkb // 2):
                    cur = issue_scores(pr, offload=pr in offl)
                    if prev is not None:
                        issue_av(pr - 1, *prev)
                    prev = cur
                issue_av(nkb // 2 - 1, *prev)
                sums = n_pool.tile([1, 512], F32, tag="sums", name="sums")
                nc.vector.tensor_copy(sums[:], A_ps[HD : HD + 1, :])
                rec = n_pool.tile([1, 512], F32, tag="rec", name="rec")
                nc.vector.reciprocal_approx_fast(rec[:], sums[:])
                bc = b_pool.tile([HD, 512], F32, name="bc")
                nc.gpsimd.partition_broadcast(bc[:], rec[0:1, :])
                nc.vector.tensor_tensor(
                    AT[pb : pb + HD, t, j * 512 : (j + 1) * 512],
                    A_ps[0:HD, :],
                    bc[:],
                    ALU.mult,
                )

            def out_proj_group(j, g, use_s_pool=False):
                st = 4 * j + g // 2
                oc = g % 2
                if use_s_pool:  # tail: rotate through the idle score banks
                    o_ps = s_psum.tile([P, 2, 512], F32, tag="s", name="o_ps")[:, 0]
                else:
                    o_ps = proj_psum.tile([P, 512], F32, tag="pp", name="o_ps")
                for t2 in range(NT):
                    nc.tensor.matmul(
                        o_ps[:],
                        lhsT=AT[:, t2, st * P : (st + 1) * P],
                        rhs=wo_sb[:, t2, oc * 512 : (oc + 1) * 512],
                        start=(t2 == 0),
                        stop=(t2 == NT - 1),
                    )
                ot = o_pool.tile([P, 512], F32, name="ot")
                nc.vector.tensor_tensor(
                    ot[:], o_ps[:], bob[:, oc * 512 : (oc + 1) * 512], ALU.add
                )
                nc.sync.dma_start(
                    out_d[st * P : (st + 1) * P, oc * 512 : (oc + 1) * 512], ot[:]
                )

            def keep_warm(n):
                kw = s_psum.tile([P, 2, 512], F32, tag="s", name="kw")
                for _ in range(n):
                    nc.tensor.matmul(
                        kw[:, 0, 0:P], lhsT=junk[:], rhs=junk[:], start=True, stop=True
                    )

            # j-chunk 0 projections up front; then pipeline: attention(j)
            # interleaved with the projections of chunk j+1 at head
            # granularity, plus chunk j-1's output projection.
            for g in range(12):
                proj_group(0, g)
            for j in range(NQ):
                filler = [("p", j + 1, g) for g in range(12)] if j + 1 < NQ else []
                if j > 0:
                    filler += [("o", j - 1, g) for g in range(8)]
                for h in range(HG):
                    attn_head(j, h)
                    k0 = (len(filler) * h) // HG
                    k1 = (len(filler) * (h + 1)) // HG
                    for kind, jj, g in filler[k0:k1]:
                        if kind == "p":
                            proj_group(jj, g)
                        else:
                            out_proj_group(jj, g)
                    if j == NQ - 1:
                        # no filler work in the last chunk: dummy matmuls keep
                        # the PE HAM clock gate warm across the exp waits and
                        # the final head's normalize chain
                        keep_warm(44 if h == HG - 1 else 2)
            # tail: the last chunk's output projection, alternating psum
            # pools so the bias-add/DMA chain never gates the matmuls
            for g in range(8):
                out_proj_group(NQ - 1, g, use_s_pool=bool(g % 2))

    nc.compile()

    return nc


def _get_nc():
    if "nc" not in _NC_CACHE:
        _NC_CACHE["nc"] = _build_nc()
    return _NC_CACHE["nc"]


def make_in_maps(x, wq, bq, wk, bk, wv, bv, wo, bo, n_cores=N_CORES):
    import ml_dtypes

    bf = ml_dtypes.bfloat16
    f8 = ml_dtypes.float8_e4m3
    P = 128
    NQ = S // 512
    x = np.asarray(x, np.float32)
    wq, wk, wv, wo = (np.asarray(a, np.float32) for a in (wq, wk, wv, wo))
    bq = np.asarray(bq, np.float32)
    bv, bo = np.asarray(bv, np.float32), np.asarray(bo, np.float32)

    # per-batch x layouts (shared by both cores of the batch)
    xf8_b, xbf_b = [], []
    for b in range(B):
        xT = np.ascontiguousarray(x[b].T)  # [D, S]
        # [p, j, c, i, s] = xT[256c+128i+p, 512j+s]
        xf8 = (
            xT.reshape(4, 2, P, NQ, 512)
            .transpose(2, 3, 0, 1, 4)
            .reshape(P, -1)
            .astype(f8)
        )
        # [p, j, d, s] = xT[128d+p, 512j+s]
        xbf = (
            xT.reshape(8, P, NQ, 512).transpose(1, 2, 0, 3).reshape(P, -1).astype(bf)
        )
        xf8_b.append(np.ascontiguousarray(xf8))
        xbf_b.append(np.ascontiguousarray(xbf))

    def wqk_f8(w, cs):
        # [p, c, i, m] = WSCALE * w[256c+128i+p, cs][., m]
        return np.ascontiguousarray(
            (w[:, cs] * WSCALE).reshape(4, 2, P, QC).transpose(2, 0, 1, 3).reshape(P, -1).astype(f8)
        )

    in_maps = []
    for c in range(n_cores):
        b, g = c // 2, c % 2
        cs = slice(g * QC, (g + 1) * QC)
        wv_l = np.ascontiguousarray(
            wv[:, cs].reshape(8, P, QC).transpose(1, 0, 2).reshape(P, -1).astype(bf)
        )
        wo_l = np.ascontiguousarray(
            wo[cs, :].reshape(4, P, D).transpose(1, 0, 2).reshape(P, -1).astype(bf)
        )
        in_maps.append(
            {
                "xf8": xf8_b[b],
                "xbf": xbf_b[b],
                "wqf8": wqk_f8(wq, cs),
                "wkf8": wqk_f8(wk, cs),
                "wv": wv_l,
                "wo": wo_l,
                "bq32": np.ascontiguousarray(bq[cs] * WSCALE),
                "bvb": np.ascontiguousarray(np.broadcast_to(bv[cs], (128, QC))),
                "bob": np.ascontiguousarray(
                    np.broadcast_to(bo if g == 0 else np.zeros_like(bo), (128, D))
                ),
            }
        )
    return in_maps


def kernel(x, wq, bq, wk, bk, wv, bv, wo, bo):
    global LAST_RESULT
    in_maps = make_in_maps(x, wq, bq, wk, bk, wv, bv, wo, bo)
    nc = _get_nc()
    trace = os.environ.get("MHA_TRACE", "0") == "1"
    res = run_bass_kernel_spmd(nc, in_maps, core_ids=list(range(N_CORES)), trace=trace)
    LAST_RESULT = res

    out = np.empty((B, S, D), np.float32)
    for b in range(B):
        out[b] = res.results[2 * b]["out"] + res.results[2 * b + 1]["out"]
    return out
